# revision 5
# baseline (speedup 1.0000x reference)
"""CondConv2d (MoE routed conv) Trainium2 kernel.

Math: out[b] = sum_e routing[b,e] * conv3x3(x[b], W[e])
Since the expert mix is linear in W, this equals
    out[b] = conv3x3(x[b], Wmix_b),  Wmix_b = sum_e routing[b,e] * W[e]
which needs 1 conv per sample instead of E=4 (4x less PE work).
The per-sample Wmix is computed on the host (numpy, fp32) during input
prep — it is tiny (B*CIN*9*COUT) and removing the on-device mix both
halves the weight DMA bytes (2 mixed sets vs 4 expert sets) and removes
the weight-DMA -> DVE-mix -> matmul dependency chain from the critical
path.

Sharding: data-parallel over batch, B=16 -> 2 samples per core on 8 cores.

Conv as implicit GEMM: x is zero-padded on host to [ci, 58, 58]; for each
of 9 taps the matmul streams a shifted window of the padded image
(rhs = xpad[:, r0+kh : r0+kh+nr, kw : kw+56]) against the tap's mixed
weight slice (lhsT = Wmix[ci, co], K=ci on partitions). Both samples run
block-outer: all 9 taps accumulate into one PSUM bank per 8-row block,
then the block is drained (DVE fp32->fp16 copy to SBUF, DMA out), so
output stores stream throughout the kernel instead of bursting at the
end. Output is fp16 on the wire (halves store bytes; ~5e-4 rel err is
far inside the 2e-2 budget) and is upcast to fp32 on the host.

Numerics: x and Wmix are rounded to fp16 on the host; matmuls run fp16
at 1 cycle/row with fp32 PSUM accumulation (~4e-4 L2 relative error).

Schedule: the PE p-state ramp needs ~3us of continuous busy time to
reach 2.4 GHz, and the first input DMA lands ~3.3us after kernel start,
so N_WARM small (N=128) dummy matmuls on a zeroed tile run first — the
PE is warm and free right as data arrives. Loads are split across the
sync and scalar DMA rings in need-time order: sync carries the
first-matmul gates (tap-0 weights, x rows 0-18) then the output stores;
scalar carries the remaining weights and x rows.
"""

import os
import sys

os.environ.setdefault("MYCRO_LOCAL_CACHE", "1")
for _p in ("/opt/trn_rl_repo",):
    if _p not in sys.path:
        sys.path.insert(0, _p)

import numpy as np

B, CIN, COUT, H, W_SP = 16, 128, 128, 56, 56
E, KH, KW = 4, 3, 3
NCORES = 8
SPC = B // NCORES          # samples per core
HP, WP = H + 2, W_SP + 2   # padded spatial
NTAP = KH * KW
RPB = 8                    # output rows per matmul block
W_TAP = COUT               # weight columns per tap

N_WARM = 7                 # HAM warm-up dummy matmuls
WARM_N = 512               # moving-operand size of each warm matmul

# sample-0 x chunks (start_row, n_rows) and block->chunk map; chunk 0
# (gates the first matmul) rides the scalar ring, head position
XCH0 = [(0, 18), (16, 42)]
BLK_CH0 = [0, 0, 1, 1, 1, 1, 1]
# row blocks; sample 1 gets a small final block to shorten the tail
BLKS0 = [(0, 8), (8, 8), (16, 8), (24, 8), (32, 8), (40, 8), (48, 8)]
BLKS1 = [(0, 8), (8, 8), (16, 8), (24, 8), (32, 8), (40, 8), (48, 6), (54, 2)]
# sample-0 weight tap chunks (start_tap, n_taps), split across the two
# rings' head slots so both descriptors process in parallel with x0c0
WTCH0 = [(0, 3), (3, 6)]

_cached_nc = None


def _build_nc():
    import concourse.tile as tile
    from concourse import bacc, mybir

    f32 = mybir.dt.float32
    f16 = mybir.dt.float16

    nc = bacc.Bacc(
        "TRN2", target_bir_lowering=False, debug=False, num_devices=NCORES
    )

    xpad_d = nc.dram_tensor(
        "xpad", [SPC, CIN, HP * WP], f16, kind="ExternalInput"
    ).ap()
    # host-mixed per-sample weights: [ci, (tap, co)]
    wm_d = nc.dram_tensor(
        "wm", [SPC, CIN, NTAP * W_TAP], f16, kind="ExternalInput"
    ).ap()
    out_d = nc.dram_tensor(
        "out", [SPC, COUT, H * W_SP], f16, kind="ExternalOutput"
    ).ap()

    with tile.TileContext(nc) as tc:
        with (
            tc.tile_pool(name="const", bufs=1) as cst,
            tc.tile_pool(name="x", bufs=1) as xpool,
            tc.tile_pool(name="ob", bufs=2) as opool,
            tc.tile_pool(name="ps", bufs=8, space="PSUM") as pspool,
        ):
            # --- input loads, in global need-time order per ring.
            # Weight chunks get separate tiles: matmul weight reads are
            # tracked whole-tile, so each chunk's matmuls gate only on
            # its own DMA.
            # ring heads in parallel: sync gets taps 0-2, scalar gets
            # x0 chunk 0; then the rest in need order
            wm0 = {}  # tap -> (tile, local tap index)
            wtiles = []
            for c, (t0, ntc) in enumerate(WTCH0):
                wt = cst.tile([CIN, ntc * W_TAP], f16, tag=f"wm0_{c}",
                              name=f"wm0_{c}")
                wtiles.append((wt, t0, ntc))
                for tt in range(t0, t0 + ntc):
                    wm0[tt] = (wt, tt - t0)

            def load_w(c, eng):
                wt, t0, ntc = wtiles[c]
                sl = slice(t0 * W_TAP, (t0 + ntc) * W_TAP)
                eng.dma_start(wt[:], wm_d[0][:, sl])

            x0t = []
            for c, (r0, nr) in enumerate(XCH0):
                xt = xpool.tile([CIN, nr * WP], f16, tag=f"x0_{c}",
                                name=f"x0_{c}")
                x0t.append(xt)

            def load_x0(c, eng):
                r0, nr = XCH0[c]
                eng.dma_start(x0t[c][:], xpad_d[0][:, r0 * WP:(r0 + nr) * WP])

            load_w(0, nc.sync)       # taps 0-2
            load_x0(0, nc.scalar)    # x rows 0-18
            load_w(1, nc.scalar)     # taps 3-8
            load_x0(1, nc.sync)      # x rows 16-58

            wm1 = cst.tile([CIN, NTAP * W_TAP], f16, tag="wm1")
            nc.scalar.dma_start(wm1[:], wm_d[1])
            x1t = xpool.tile([CIN, HP * WP], f16, tag="x1")
            nc.scalar.dma_start(x1t[:], xpad_d[1])

            # --- HAM warm-up: dummy matmuls on a zeroed tile during loads
            zt = cst.tile([128, WARM_N], f16, tag="zero")
            nc.gpsimd.memset(zt[:], 0.0)
            warm_ps = pspool.tile([128, WARM_N], f32, tag="ps")
            for _ in range(N_WARM):
                nc.tensor.matmul(
                    warm_ps[:], zt[:, :128], zt[:], start=True, stop=True
                )

            def rhs_ap(xt, xoff, r0, nr, kh, kw):
                x3 = xt[:].rearrange("p (h w) -> p h w", w=WP)
                loc = r0 - xoff
                return x3[:, loc + kh : loc + kh + nr, kw : kw + W_SP]

            def sample(s, blks, wmap, xfor, ob):
                for blk, (r0, nr) in enumerate(blks):
                    ps = pspool.tile(
                        [COUT, nr * W_SP], f32, tag="ps", name=f"ps{s}_{blk}"
                    )
                    for t in range(NTAP):
                        kh, kw = divmod(t, KW)
                        wt, loc = wmap[t]
                        xt, xoff = xfor(blk)
                        nc.tensor.matmul(
                            ps[:],
                            wt[:, loc * W_TAP : (loc + 1) * W_TAP],
                            rhs_ap(xt, xoff, r0, nr, kh, kw),
                            start=(t == 0),
                            stop=(t == NTAP - 1),
                        )
                    sl = slice(r0 * W_SP, (r0 + nr) * W_SP)
                    nc.vector.tensor_copy(ob[:, sl], ps[:])
                    nc.sync.dma_start(out_d[s][:, sl], ob[:, sl])

            ob0 = opool.tile([COUT, H * W_SP], f16, tag="ob")
            sample(
                0, BLKS0, wm0,
                lambda blk: (x0t[BLK_CH0[blk]], XCH0[BLK_CH0[blk]][0]), ob0,
            )

            wm1map = {t: (wm1, t) for t in range(NTAP)}
            ob1 = opool.tile([COUT, H * W_SP], f16, tag="ob")
            sample(1, BLKS1, wm1map, lambda blk: (x1t, 0), ob1)

    nc.compile()
    return nc


def _get_nc():
    global _cached_nc
    if _cached_nc is None:
        _cached_nc = _build_nc()
    return _cached_nc


def _prep_inputs(x, routing_weights, W):
    x = np.ascontiguousarray(x, dtype=np.float32)
    routing_weights = np.ascontiguousarray(routing_weights, dtype=np.float32)
    W = np.ascontiguousarray(W, dtype=np.float32)

    xpad = np.zeros((B, CIN, HP, WP), np.float16)
    xpad[:, :, 1 : H + 1, 1 : W_SP + 1] = x.reshape(B, CIN, H, W_SP)
    xpad = xpad.reshape(B, CIN, HP * WP)

    # W[e, co, ci, kh, kw] -> Wt[e, ci, (kh kw), co]; host mix over experts
    Wt = np.transpose(W, (0, 2, 3, 4, 1)).reshape(E, CIN, NTAP, COUT)
    wmix = np.tensordot(routing_weights, Wt, axes=(1, 0))  # [B, ci, tap, co]
    wm16 = wmix.reshape(B, CIN, NTAP * W_TAP).astype(np.float16)

    in_maps = []
    for c in range(NCORES):
        in_maps.append(
            {
                "xpad": np.ascontiguousarray(xpad[c * SPC : (c + 1) * SPC]),
                "wm": np.ascontiguousarray(wm16[c * SPC : (c + 1) * SPC]),
            }
        )
    return in_maps


def _run(in_maps, **kwargs):
    from concourse import bass_utils

    nc = _get_nc()
    res = bass_utils.run_bass_kernel_spmd(
        nc, in_maps, core_ids=list(range(NCORES)), **kwargs
    )
    out = np.concatenate(
        [res.results[c]["out"] for c in range(NCORES)], axis=0
    ).reshape(B, COUT, H, W_SP).astype(np.float32)
    return out, res


def kernel(x, routing_weights, W):
    in_maps = _prep_inputs(x, routing_weights, W)
    out, _ = _run(in_maps)
    return out


# revision 7
# speedup vs baseline: 1.2057x; 1.2057x over previous
"""CondConv2d (MoE routed conv) Trainium2 kernel.

Math: out[b] = sum_e routing[b,e] * conv3x3(x[b], W[e])
Since the expert mix is linear in W, this equals
    out[b] = conv3x3(x[b], Wmix_b),  Wmix_b = sum_e routing[b,e] * W[e]
which needs 1 conv per sample instead of E=4 (4x less PE work).
The per-sample Wmix is computed on the host (numpy, fp32) during input
prep — it is tiny (B*CIN*9*COUT) and removing the on-device mix both
halves the weight DMA bytes (2 mixed sets vs 4 expert sets) and removes
the weight-DMA -> DVE-mix -> matmul dependency chain from the critical
path.

Sharding: data-parallel over batch, B=16 -> 2 samples per core on 8 cores.

Conv as implicit GEMM: x is zero-padded on host to [ci, 58, 58]; for each
of 9 taps the matmul streams a shifted window of the padded image
(rhs = xpad[:, r0+kh : r0+kh+nr, kw : kw+56]) against the tap's mixed
weight slice (lhsT = Wmix[ci, co], K=ci on partitions). Both samples run
block-outer: all 9 taps accumulate into one PSUM bank per 8-row block,
then the block is drained (DVE fp32->fp16 copy to SBUF, DMA out), so
output stores stream throughout the kernel instead of bursting at the
end. Output is fp16 on the wire (halves store bytes; ~5e-4 rel err is
far inside the 2e-2 budget) and is upcast to fp32 on the host.

Numerics: x and Wmix are rounded to fp16 on the host; matmuls run fp16
at 1 cycle/row with fp32 PSUM accumulation (~4e-4 L2 relative error).

Schedule: the PE p-state ramp needs ~3us of continuous busy time to
reach 2.4 GHz, and the first input DMA lands ~3.3us after kernel start,
so N_WARM small (N=128) dummy matmuls on a zeroed tile run first — the
PE is warm and free right as data arrives. Loads are split across the
sync and scalar DMA rings in need-time order: sync carries the
first-matmul gates (tap-0 weights, x rows 0-18) then the output stores;
scalar carries the remaining weights and x rows.
"""

import os
import sys

os.environ.setdefault("MYCRO_LOCAL_CACHE", "1")
for _p in ("/opt/trn_rl_repo",):
    if _p not in sys.path:
        sys.path.insert(0, _p)

import numpy as np

B, CIN, COUT, H, W_SP = 16, 128, 128, 56, 56
E, KH, KW = 4, 3, 3
NCORES = 8
SPC = B // NCORES          # samples per core
HP, WP = H + 2, W_SP + 2   # padded spatial
NTAP = KH * KW
RPB = 8                    # output rows per matmul block
W_TAP = COUT               # weight columns per tap

N_WARM = 7                 # HAM warm-up dummy matmuls
WARM_N = 512               # moving-operand size of each warm matmul

# sample-0 x chunks (start_row, n_rows) and block->chunk map. The DMA
# rings deliver roughly in order at ~200GB/s aggregate each, so the
# first-matmul gates (tap-0 weights + x rows 0-18) lead the sync ring
# and the remaining tap weights lead the scalar ring, ahead of the
# bulkier x transfers.
XCH0 = [(0, 18), (16, 42)]
BLK_CH0 = [0, 0, 1, 1, 1, 1, 1]
# row blocks; sample 1 gets a small final block to shorten the tail
BLKS0 = [(0, 8), (8, 8), (16, 8), (24, 8), (32, 8), (40, 8), (48, 8)]
BLKS1 = [(0, 8), (8, 8), (16, 8), (24, 8), (32, 8), (40, 8), (48, 6), (54, 2)]
# sample-0 weight tap chunks (start_tap, n_taps)
WTCH0 = [(0, 1), (1, 4), (5, 4)]

_cached_nc = None


def _build_nc():
    import concourse.tile as tile
    from concourse import bacc, mybir

    f32 = mybir.dt.float32
    f16 = mybir.dt.float16

    nc = bacc.Bacc(
        "TRN2", target_bir_lowering=False, debug=False, num_devices=NCORES
    )

    xpad_d = nc.dram_tensor(
        "xpad", [SPC, CIN, HP * WP], f16, kind="ExternalInput"
    ).ap()
    # host-mixed per-sample weights: [ci, (tap, co)]
    wm_d = nc.dram_tensor(
        "wm", [SPC, CIN, NTAP * W_TAP], f16, kind="ExternalInput"
    ).ap()
    out_d = nc.dram_tensor(
        "out", [SPC, COUT, H * W_SP], f16, kind="ExternalOutput"
    ).ap()

    with tile.TileContext(nc) as tc:
        with (
            tc.tile_pool(name="const", bufs=1) as cst,
            tc.tile_pool(name="x", bufs=1) as xpool,
            tc.tile_pool(name="ob", bufs=2) as opool,
            tc.tile_pool(name="ps", bufs=8, space="PSUM") as pspool,
        ):
            # --- input loads, in global need-time order per ring.
            # Weight chunks get separate tiles: matmul weight reads are
            # tracked whole-tile, so each chunk's matmuls gate only on
            # its own DMA.
            # ring heads in parallel: sync gets taps 0-2, scalar gets
            # x0 chunk 0; then the rest in need order
            wm0 = {}  # tap -> (tile, local tap index)
            wtiles = []
            for c, (t0, ntc) in enumerate(WTCH0):
                wt = cst.tile([CIN, ntc * W_TAP], f16, tag=f"wm0_{c}",
                              name=f"wm0_{c}")
                wtiles.append((wt, t0, ntc))
                for tt in range(t0, t0 + ntc):
                    wm0[tt] = (wt, tt - t0)

            def load_w(c, eng):
                wt, t0, ntc = wtiles[c]
                sl = slice(t0 * W_TAP, (t0 + ntc) * W_TAP)
                eng.dma_start(wt[:], wm_d[0][:, sl])

            x0t = []
            for c, (r0, nr) in enumerate(XCH0):
                xt = xpool.tile([CIN, nr * WP], f16, tag=f"x0_{c}",
                                name=f"x0_{c}")
                x0t.append(xt)

            def load_x0(c, eng):
                r0, nr = XCH0[c]
                eng.dma_start(x0t[c][:], xpad_d[0][:, r0 * WP:(r0 + nr) * WP])

            load_w(0, nc.sync)       # tap 0 (gates the first matmul)
            load_x0(0, nc.sync)      # x rows 0-18 (gates the first matmul)
            load_w(1, nc.scalar)     # taps 1-4
            load_w(2, nc.scalar)     # taps 5-8
            load_x0(1, nc.scalar)    # x rows 16-58

            wm1 = cst.tile([CIN, NTAP * W_TAP], f16, tag="wm1")
            nc.scalar.dma_start(wm1[:], wm_d[1])
            x1t = xpool.tile([CIN, HP * WP], f16, tag="x1")
            nc.scalar.dma_start(x1t[:], xpad_d[1])

            # --- HAM warm-up: dummy matmuls on a zeroed tile during loads
            zt = cst.tile([128, WARM_N], f16, tag="zero")
            nc.gpsimd.memset(zt[:], 0.0)
            warm_ps = pspool.tile([128, WARM_N], f32, tag="ps")
            for _ in range(N_WARM):
                nc.tensor.matmul(
                    warm_ps[:], zt[:, :128], zt[:], start=True, stop=True
                )

            def rhs_ap(xt, xoff, r0, nr, kh, kw):
                x3 = xt[:].rearrange("p (h w) -> p h w", w=WP)
                loc = r0 - xoff
                return x3[:, loc + kh : loc + kh + nr, kw : kw + W_SP]

            def sample(s, blks, wmap, xfor, ob):
                for blk, (r0, nr) in enumerate(blks):
                    ps = pspool.tile(
                        [COUT, nr * W_SP], f32, tag="ps", name=f"ps{s}_{blk}"
                    )
                    for t in range(NTAP):
                        kh, kw = divmod(t, KW)
                        wt, loc = wmap[t]
                        xt, xoff = xfor(blk)
                        nc.tensor.matmul(
                            ps[:],
                            wt[:, loc * W_TAP : (loc + 1) * W_TAP],
                            rhs_ap(xt, xoff, r0, nr, kh, kw),
                            start=(t == 0),
                            stop=(t == NTAP - 1),
                        )
                    sl = slice(r0 * W_SP, (r0 + nr) * W_SP)
                    nc.vector.tensor_copy(ob[:, sl], ps[:])
                    nc.sync.dma_start(out_d[s][:, sl], ob[:, sl])

            ob0 = opool.tile([COUT, H * W_SP], f16, tag="ob")
            sample(
                0, BLKS0, wm0,
                lambda blk: (x0t[BLK_CH0[blk]], XCH0[BLK_CH0[blk]][0]), ob0,
            )

            wm1map = {t: (wm1, t) for t in range(NTAP)}
            ob1 = opool.tile([COUT, H * W_SP], f16, tag="ob")
            sample(1, BLKS1, wm1map, lambda blk: (x1t, 0), ob1)

    nc.compile()
    return nc


def _get_nc():
    global _cached_nc
    if _cached_nc is None:
        _cached_nc = _build_nc()
    return _cached_nc


def _prep_inputs(x, routing_weights, W):
    x = np.ascontiguousarray(x, dtype=np.float32)
    routing_weights = np.ascontiguousarray(routing_weights, dtype=np.float32)
    W = np.ascontiguousarray(W, dtype=np.float32)

    xpad = np.zeros((B, CIN, HP, WP), np.float16)
    xpad[:, :, 1 : H + 1, 1 : W_SP + 1] = x.reshape(B, CIN, H, W_SP)
    xpad = xpad.reshape(B, CIN, HP * WP)

    # W[e, co, ci, kh, kw] -> Wt[e, ci, (kh kw), co]; host mix over experts
    Wt = np.transpose(W, (0, 2, 3, 4, 1)).reshape(E, CIN, NTAP, COUT)
    wmix = np.tensordot(routing_weights, Wt, axes=(1, 0))  # [B, ci, tap, co]
    wm16 = wmix.reshape(B, CIN, NTAP * W_TAP).astype(np.float16)

    in_maps = []
    for c in range(NCORES):
        in_maps.append(
            {
                "xpad": np.ascontiguousarray(xpad[c * SPC : (c + 1) * SPC]),
                "wm": np.ascontiguousarray(wm16[c * SPC : (c + 1) * SPC]),
            }
        )
    return in_maps


def _run(in_maps, **kwargs):
    from concourse import bass_utils

    nc = _get_nc()
    res = bass_utils.run_bass_kernel_spmd(
        nc, in_maps, core_ids=list(range(NCORES)), **kwargs
    )
    out = np.concatenate(
        [res.results[c]["out"] for c in range(NCORES)], axis=0
    ).reshape(B, COUT, H, W_SP).astype(np.float32)
    return out, res


def kernel(x, routing_weights, W):
    in_maps = _prep_inputs(x, routing_weights, W)
    out, _ = _run(in_maps)
    return out
